# revision 10
# baseline (speedup 1.0000x reference)
"""Trainium2 Bass kernel for Conv2D (1x1) multi-head attention block.

Reference computation (per batch image of [64, 64, 512] = [N=4096, C=512]):
    x  = GroupNorm(inputs, G=32, eps=1e-6) * gamma + beta
    q, k, v = x @ wq + bq, x @ wk + bk, x @ wv + bv      (1x1 convs)
    scores  = (q / sqrt(C)) @ k^T                         [N, N]
    out     = softmax(scores) @ v @ wo + bo + inputs

Sharding: 8 cores = 2 batches x 4 query-quarters.  Each core holds the full
image of its batch (full-attention K/V) and produces the output rows of its
query quarter.  No collectives.

Division of labor: the host does all input-independent weight algebra plus
the GroupNorm statistics (a 2x32-number reduction) and precision/layout
prep; the device runs every activation GEMM: the query projection
u = W2^T (a.x_q) + c2 (W2 = Wq Wk^T), the full N x N attention
(scores, softmax, attn.V), and the output projection via W3 = Wv Wo.

  - GroupNorm folds: a = gamma*rstd, b = beta - mean*a.  The host ships
    xa = a.x pre-scaled and cast to fp8_e4m3 in BOTH layouts the PE needs:
    channel-pair tiles xat8 [128, 2, N] (scores lhsT / stats-free) and
    pixel-pair tiles xnat8 (attn.V lhsT).  All additive GN/bias terms either
    cancel in softmax (key-side constants), fold into c2 (query side,
    c2 = Wk (Wq^T b + bq)), or ride through attention as constants into the
    residual (V side: res16 = fp16(x + (b Wv + bv) Wo + bo)).
  - Every matmul runs in fp8 DoubleRow mode (256-deep contraction per
    instruction).  Weights ship as fp8 scaled by S (=16); the 1/S is
    recovered in PSUM->SBUF epilogues.
  - Scores are computed per 128-key tile as scores^T[k, q]; exp runs on the
    scalar engine with scale 1/sqrt(C) and bias -2 (softmax shift
    invariance; keeps exp outputs inside fp8's +-240 range) writing fp8
    probs pairs.  attn^T accumulates over key-pair tiles in PSUM; softmax
    denominators come from a DoubleRow ones-matmul into a [32, 512] PSUM
    tile.  The kernel is software-pipelined: attnV of pair g-1 issues
    between the scores and exps of pair g, so the PE never waits on the
    scalar engine.
  - V is never materialized and neither is attn: y = xa^T probs^T
    accumulates in PSUM, then out rows = (ATS.y)^T (S.W3) with
    W3 = Wv Wo host-folded to fp8 -- one GEMM instead of two, and one
    fewer fp8 requantization.  The 1/rowsum is applied per-partition after
    the output projection on the DVE (the scalar engine does only exps, so
    its Exp table is never evicted).
  - The previous chunk's epilogue matmuls (rowsum transposes + output
    projection) are interleaved into the next chunk's score stream at the
    points where the PE would otherwise wait, so chunk boundaries cost no
    PE bubble; the final chunk interleaves f32r warm matmuls instead to
    keep the HAM clock up through the fin/DMA drain.
"""

import sys

sys.path.insert(0, "/opt/trn_rl_repo")

from contextlib import ExitStack

import numpy as np

import concourse.bacc as bacc
import concourse.tile as tile
from concourse import mybir
from concourse.bass_utils import run_bass_kernel_spmd

# Problem shape (hardcoded; kernel.py must be self-contained).
B, HH, WW, C = 2, 64, 64, 512
N = HH * WW          # 4096 pixels per image
G = 32               # groupnorm groups
GS = C // G          # 16 channels per group
EPS = 1e-6
P = 128              # partitions
CT = C // P          # 4 channel tiles
CP = CT // 2         # 2 channel-pair tiles
NT = N // P          # 32 pixel tiles per image
NP2 = NT // 2        # 16 pixel-pair tiles
NCORES = 8
QS = N // 4          # 1024 query rows per core
QTILES = QS // P     # 8 query tiles per core
QCH = QS // 512      # 2 query chunks per core

S2 = 16.0            # fp8 scale for W2 = Wq @ Wk^T (host-precomputed)
S3 = 16.0            # fp8 scale for W3 = Wv @ Wo (host-precomputed)
ATS = 0.125          # unnormalized-attn fp8 scale (|attn_u| < ~800 -> <100)
ISQ = 1.0 / float(np.sqrt(float(C)))
SHIFT = -2.0         # exp(s*ISQ + SHIFT): keeps probs < 240 (fp8e4 max)

F32 = mybir.dt.float32
F16 = mybir.dt.float16
BF16 = mybir.dt.bfloat16
FP8 = mybir.dt.float8e4
AF = mybir.ActivationFunctionType
ALU = mybir.AluOpType
DR = mybir.MatmulPerfMode.DoubleRow

_NC_CACHE = None


def _build():
    nc = bacc.Bacc(None, target_bir_lowering=False, debug=False)

    xat8_d = [nc.dram_tensor(f"xat8p{g}", [P, 2, N], FP8, kind="ExternalInput")
              for g in range(CP)]
    xnat_d = nc.dram_tensor("xnat8", [NP2, P, 2, C], FP8, kind="ExternalInput")
    # query-quarter columns of xat, pairs stacked along dim 1 as (g, i) -> 2g+i
    xaq8_d = nc.dram_tensor("xaq8", [P, 2 * CP, QS], FP8, kind="ExternalInput")
    w2T8_d = [nc.dram_tensor(f"w2T8p{g}", [P, 2, C], FP8, kind="ExternalInput")
              for g in range(CP)]
    w38_d = [nc.dram_tensor(f"w38p{g}", [P, 2, C], FP8, kind="ExternalInput")
             for g in range(CP)]
    c2_d = nc.dram_tensor("c2", [C], F32, kind="ExternalInput")
    res_d = nc.dram_tensor("res16", [QS, C], F16, kind="ExternalInput")
    one_d = nc.dram_tensor("one11", [1, 1], F32, kind="ExternalInput")
    out_d = nc.dram_tensor("out", [QS, C], BF16, kind="ExternalOutput")

    with tile.TileContext(nc) as tc, ExitStack() as top:
        consts = top.enter_context(tc.tile_pool(name="consts", bufs=1))
        pxt = top.enter_context(tc.tile_pool(name="pxt", bufs=1))
        pv = top.enter_context(tc.tile_pool(name="pv", bufs=1))
        pq = top.enter_context(tc.tile_pool(name="pq", bufs=1))
        pres = top.enter_context(tc.tile_pool(name="pres", bufs=1))
        pmisc = top.enter_context(tc.tile_pool(name="pmisc", bufs=1))
        pe = top.enter_context(tc.tile_pool(name="pe", bufs=4))
        pef = top.enter_context(tc.tile_pool(name="pef", bufs=2))
        # PSUM: sc 2 + at 4 + rows 1 + ops 1 = 8 banks
        pss = top.enter_context(tc.tile_pool(name="pss", bufs=2, space="PSUM"))
        psat = top.enter_context(tc.tile_pool(name="psat", bufs=1, space="PSUM"))
        psr = top.enter_context(tc.tile_pool(name="psr", bufs=1, space="PSUM"))
        pso = top.enter_context(tc.tile_pool(name="pso", bufs=1, space="PSUM"))

        # ---------- consts (no DMA dependencies) ----------
        one11 = consts.tile([1, 1], F32, name="one11")
        nc.sync.dma_start(out=one11, in_=one_d[:])
        ones16 = consts.tile([P, 32], F16, name="ones16")
        nc.vector.memset(ones16, 1.0)
        ebias = consts.tile([P, 1], F32, name="ebias")
        nc.vector.memset(ebias, SHIFT)
        warm32 = pmisc.tile([P, 512], F32, name="warm32")
        nc.vector.memset(warm32, 1.0)
        warmr = pmisc.tile([P, 512], mybir.dt.float32r, name="warmr")
        nc.vector.tensor_copy(warmr, warm32)

        # ---------- resident tensors ----------
        xat8 = [pxt.tile([P, 2, N], FP8, name=f"xat8_{g}", tag=f"xat8_{g}")
                for g in range(CP)]
        xnat = [pv.tile([P, 2, C], FP8, name=f"xnat_{g}", tag=f"xnat_{g}")
                for g in range(NP2)]
        xaq8 = pq.tile([P, 2 * CP, QS], FP8, name="xaq8", tag="xaq8")
        u8 = [pq.tile([P, 2, QS], FP8, name=f"u8_{g}", tag=f"u8_{g}")
              for g in range(CP)]
        w2T8 = [pq.tile([P, 2, C], FP8, name=f"w2T8_{g}", tag=f"w2T8_{g}")
                for g in range(CP)]
        w38 = [pq.tile([P, 2, C], FP8, name=f"w38_{g}", tag=f"w38_{g}")
               for g in range(CP)]
        res16 = [pres.tile([P, C], F16, name=f"res16_{i}", tag=f"res_{i}")
                 for i in range(QTILES)]
        c24 = []
        for ct in range(CT):
            c_t = consts.tile([P, 1], F32, name=f"c24_{ct}")
            nc.sync.dma_start(out=c_t, in_=c2_d[ct * P:(ct + 1) * P])
            c24.append(c_t)

        # ---------- DMA issue order: the ~16 hardware queues are assigned
        # round-robin in call order and run CONCURRENTLY, so a transfer's
        # priority is its share of queues, not its position.  Stripe the
        # u8-projection dependencies (w2T8 + xaq8 chunk 0) into 12 slices so
        # they own most of the queues and land first; everything else
        # follows in consumption order, residuals last. ----------
        for g in range(CP):
            for s in range(4):
                nc.sync.dma_start(out=w2T8[g][:, :, s * 128:(s + 1) * 128],
                                  in_=w2T8_d[g][:, :, s * 128:(s + 1) * 128])
        for s in range(4):
            nc.sync.dma_start(out=xaq8[:, :, s * 128:(s + 1) * 128],
                              in_=xaq8_d[:, :, s * 128:(s + 1) * 128])
        nc.sync.dma_start(out=xaq8[:, :, 512:1024],
                          in_=xaq8_d[:, :, 512:1024])
        for g in range(CP):
            nc.sync.dma_start(out=w38[g], in_=w38_d[g][:])
        for ch in range(8):          # 512-pixel column chunks, kt-major
            c0, c1 = ch * 512, (ch + 1) * 512
            for g in range(CP):
                nc.sync.dma_start(out=xat8[g][:, :, c0:c1],
                                  in_=xat8_d[g][:, :, c0:c1])
            nc.sync.dma_start(out=xnat[2 * ch], in_=xnat_d[2 * ch])
            nc.sync.dma_start(out=xnat[2 * ch + 1], in_=xnat_d[2 * ch + 1])
        for i in range(QTILES):
            nc.sync.dma_start(out=res16[i], in_=res_d[i * P:(i + 1) * P, :])

        def keep_warm(n):
            # Full-width f32r matmuls keep the HAM clock at full rate while
            # the PE would otherwise idle (low-toggle fp8 matmuls don't
            # register enough activity and the whole core drops to half
            # clock, with ~10us of hysteresis).
            for _ in range(n):
                wps = pss.tile([P, 512], F32, name="wps", tag="sc")
                nc.tensor.matmul(wps, lhsT=warmr[:, 0:P], rhs=warmr,
                                 start=True, stop=True)

        keep_warm(5)

        # Preload the scalar engine's Exp activation table (costs 1.3us;
        # otherwise it lands at the first real exp, right at attention
        # start).  The scalar engine runs nothing but Exp, so the table is
        # never evicted.
        expwarm = pmisc.tile([P, 1], F32, name="expwarm")
        nc.scalar.activation(expwarm, ebias, AF.Exp, bias=ebias, scale=1.0)

        # ---- query projection: u = W2^T xa_q / S2 + c2, fp8 ----
        # ps lives in the pso bank: chunk 1's projections are injected into
        # chunk 0's score stream, and borrowing the sc rotation there would
        # stall the scores on the exp reads.
        def u8_proj(ch2, ci_t):
            ps = pso.tile([P, 512], F32, name="ups", tag="ops")
            for gq in range(CP):
                nc.tensor.matmul(
                    ps,
                    lhsT=w2T8[gq][:, :, ci_t * P:(ci_t + 1) * P],
                    rhs=xaq8[:, 2 * gq:2 * gq + 2,
                             ch2 * 512:(ch2 + 1) * 512],
                    start=(gq == 0), stop=(gq == CP - 1),
                    perf_mode=DR)
            og, oi = divmod(ci_t, 2)
            nc.vector.tensor_scalar(
                u8[og][:, oi, ch2 * 512:(ch2 + 1) * 512],
                ps, 1.0 / S2, c24[ci_t], ALU.mult, ALU.add)

        for ci_t in range(CT):
            u8_proj(0, ci_t)

        # ---- attention + output projection ----
        # ep_carry: thunks of deferred PE/DVE work (the previous chunk's
        # epilogue, or the second chunk's query projection) injected one
        # per score group so the in-order PE queue never stalls on the
        # DVE-paced epilogue.  The chunk's first group has no attn_v, so
        # multi-matmul thunks land there for free.
        ep_carry = [(lambda ci_t=ci_t: u8_proj(1, ci_t)) for ci_t in range(CT)]

        def attn_v(g, probs, at_ps):
            for co in range(CT):
                nc.tensor.matmul(
                    at_ps[co],
                    lhsT=xnat[g][:, :, co * P:(co + 1) * P],
                    rhs=probs,
                    start=(g == 0), stop=(g == NP2 - 1),
                    perf_mode=DR)

        for qc in range(QCH):
            at_ps = [psat.tile([P, 512], F32, name=f"at{co}",
                               tag=f"at{co}") for co in range(CT)]
            # Two probs accumulators: even groups on the DVE, odd groups on
            # the (otherwise idle) GPSIMD, so neither engine backs up and
            # stalls the probs-buffer rotation.
            acc_d = pe.tile([P, 2, 512], F16, name="acc_d", tag="acc_d")
            acc_p = pe.tile([P, 2, 512], F16, name="acc_p", tag="acc_p")

            prev = None
            for g in range(NP2):
                scs = []
                for j in range(2):
                    kt_i = 2 * g + j
                    sc = pss.tile([P, 512], F32, name="sc", tag="sc")
                    for c in range(CP):
                        nc.tensor.matmul(
                            sc,
                            lhsT=xat8[c][:, :, kt_i * P:(kt_i + 1) * P],
                            rhs=u8[c][:, :, qc * 512:(qc + 1) * 512],
                            start=(c == 0), stop=(c == CP - 1),
                            perf_mode=DR)
                    scs.append(sc)
                if prev is not None:
                    attn_v(g - 1, prev, at_ps)
                if ep_carry:
                    ep_carry.pop(0)()
                probs = pe.tile([P, 2, 512], FP8, name="probs", tag="probs")
                for j in range(2):
                    nc.scalar.activation(probs[:, j, :], scs[j], AF.Exp,
                                         bias=ebias, scale=ISQ)
                # softmax denominators: accumulate probs off the PE; the
                # partition reduction happens once per chunk in the
                # epilogue.
                eng, acc = (nc.vector, acc_d) if g % 2 == 0 else \
                           (nc.gpsimd, acc_p)
                if g < 2:
                    eng.tensor_copy(acc, probs)
                else:
                    eng.tensor_add(acc, acc, probs)
                prev = probs
            attn_v(NP2 - 1, prev, at_ps)

            # ---- chunk epilogue ----
            rows_ps = psr.tile([32, 512], F32, name="rows", tag="rows")
            for ai, acc in enumerate((acc_d, acc_p)):
                for j in range(2):
                    nc.tensor.matmul(rows_ps, lhsT=ones16, rhs=acc[:, j, :],
                                     start=(ai == 0 and j == 0),
                                     stop=(ai == 1 and j == 1))
            # DVE (in-order): rows_sb frees the rows bank, z8 frees the at
            # banks, then recq / fin chase the PE's transposes / out-projs.
            rows_sb = pe.tile([1, 512], F32, name="rows_sb", tag="rows_sb")
            nc.vector.tensor_scalar_mul(rows_sb, rows_ps[0:1, :], ATS * S3)
            z8 = [pe.tile([P, 2, 512], FP8, name=f"z8_{zg}", tag=f"z8_{zg}")
                  for zg in range(CP)]
            for ci_t in range(CT):
                og, oi = divmod(ci_t, 2)
                nc.vector.tensor_scalar_mul(z8[og][:, oi, :],
                                            at_ps[ci_t], ATS)

            recq4 = pe.tile([P, 4], F32, name="recq4", tag="recq4")

            def mk_transp(rows_sb=rows_sb, recq4=recq4):
                def thunk():
                    rq_ps = pso.tile([P, 8], F32, name="rq_ps", tag="ops")
                    for qt in range(4):
                        nc.tensor.matmul(
                            rq_ps[:, qt:qt + 1],
                            lhsT=rows_sb[0:1, qt * P:(qt + 1) * P],
                            rhs=one11, start=True, stop=True)
                    nc.vector.reciprocal(recq4, rq_ps[:, 0:4])
                return thunk

            def mk_oproj(qt, qc=qc, z8=z8, recq4=recq4, opool=None):
                def thunk():
                    if opool is None:
                        ops = pso.tile([P, C], F32, name="ops", tag="ops")
                    else:
                        ops = opool[0].tile([P, C], F32, name="ops",
                                            tag=opool[1])
                    for zg in range(CP):
                        nc.tensor.matmul(
                            ops, lhsT=z8[zg][:, :, qt * P:(qt + 1) * P],
                            rhs=w38[zg], start=(zg == 0),
                            stop=(zg == CP - 1), perf_mode=DR)
                    fin = pef.tile([P, C], F32, name="fin", tag="fin")
                    nc.vector.tensor_scalar_mul(fin, ops, recq4[:, qt:qt + 1])
                    fin2 = pef.tile([P, C], BF16, name="fin2", tag="fin2")
                    nc.vector.tensor_add(fin2, fin, res16[qc * 4 + qt])
                    r0 = (qc * 4 + qt) * P
                    nc.sync.dma_start(out=out_d[r0:r0 + P, :], in_=fin2)
                return thunk

            if qc < QCH - 1:
                ep_carry = [mk_transp()] + [mk_oproj(qt) for qt in range(4)]
            else:
                # Last chunk: no next score stream to hide behind.  Spread
                # the out-projections over the now-free sc/at banks so they
                # run back-to-back, with warm matmuls keeping the clock up
                # through the fin/DMA drain.
                mk_transp()()
                keep_warm(2)
                mk_oproj(0)()
                mk_oproj(1, opool=(pss, "sc"))()
                keep_warm(2)
                mk_oproj(2, opool=(pss, "sc"))()
                mk_oproj(3, opool=(psat, "at0"))()
                keep_warm(12)

    nc.compile()
    return nc


def _make_in_maps(inputs):
    import ml_dtypes
    FP8NP = ml_dtypes.float8_e4m3
    x = np.ascontiguousarray(np.asarray(inputs["inputs"], dtype=np.float32))
    xf = x.reshape(B, N, C)
    gamma = np.asarray(inputs["gn_gamma"], np.float32)
    beta = np.asarray(inputs["gn_beta"], np.float32)
    wq = np.asarray(inputs["wq"], np.float32)
    wk = np.asarray(inputs["wk"], np.float32)
    wv = np.asarray(inputs["wv"], np.float32)
    wo = np.asarray(inputs["wo"], np.float32)
    bq = np.asarray(inputs["bq"], np.float32)
    bv = np.asarray(inputs["bv"], np.float32)
    bo = np.asarray(inputs["bo"], np.float32)

    shared = {"one11": np.ones((1, 1), np.float32)}
    w2T = (wq @ wk.T) * S2
    w2T8p = w2T.astype(FP8NP).reshape(CP, 2, P, C).transpose(0, 2, 1, 3)
    for g in range(CP):
        shared[f"w2T8p{g}"] = np.ascontiguousarray(w2T8p[g])
    w3 = (wv @ wo) * S3
    w38p = w3.astype(FP8NP).reshape(CP, 2, P, C).transpose(0, 2, 1, 3)
    for g in range(CP):
        shared[f"w38p{g}"] = np.ascontiguousarray(w38p[g])

    # Per-batch GroupNorm folds.
    per_b = []
    for b in range(B):
        xg = xf[b].reshape(N, G, GS)
        mean = xg.mean(axis=(0, 2))
        var = xg.var(axis=(0, 2))
        a = (gamma.reshape(G, GS) / np.sqrt(var[:, None] + EPS)).reshape(C)
        bvec = beta - np.repeat(mean, GS) * a
        xa = xf[b] * a                               # [N, C]
        xa8 = xa.astype(FP8NP)
        # channel-pair tiles: xat8p[g][p, i, n] = xa^T[g*256 + i*128 + p, n]
        xaT = np.ascontiguousarray(xa8.T)            # [C, N] fp8
        xat_pairs = [np.ascontiguousarray(
            xaT.reshape(CP, 2, P, N)[g]).transpose(1, 0, 2)
            for g in range(CP)]
        xat_pairs = [np.ascontiguousarray(t) for t in xat_pairs]
        # pixel-pair tiles: xnat8[gk][p, ik, c]
        xnat = np.ascontiguousarray(
            xa8.reshape(NP2, 2, P, C).transpose(0, 2, 1, 3))
        c2v = wk @ (bvec @ wq + bq)                  # [C]
        bo_eff = (bvec @ wv + bv) @ wo + bo          # [C]
        per_b.append((xat_pairs, xnat, c2v.astype(np.float32), bo_eff))

    in_maps = []
    for core in range(NCORES):
        b, qq = divmod(core, 4)
        xat_pairs, xnat, c2v, bo_eff = per_b[b]
        m = dict(shared)
        for g in range(CP):
            m[f"xat8p{g}"] = xat_pairs[g]
        m["xnat8"] = xnat
        m["c2"] = c2v
        xq = np.concatenate(
            [xat_pairs[g][:, :, qq * QS:(qq + 1) * QS] for g in range(CP)],
            axis=1)
        m["xaq8"] = np.ascontiguousarray(xq)
        m["res16"] = np.ascontiguousarray(
            (xf[b, qq * QS:(qq + 1) * QS, :] + bo_eff).astype(np.float16))
        in_maps.append(m)
    return in_maps


def _assemble(results):
    out = np.empty((B, N, C), dtype=np.float32)
    for core in range(NCORES):
        b, qq = divmod(core, 4)
        out[b, qq * QS:(qq + 1) * QS, :] = results[core]["out"]
    return out.reshape(B, HH, WW, C)


def kernel(**inputs):
    global _NC_CACHE
    if _NC_CACHE is None:
        _NC_CACHE = _build()
    in_maps = _make_in_maps(inputs)
    res = run_bass_kernel_spmd(_NC_CACHE, in_maps, list(range(NCORES)))
    return _assemble(res.results)


def _install_ntff_shim():
    """The agent image's antenv lacks axon_hooks; provide it so
    run_bass_kernel_spmd(trace=True) can NTFF-profile through axon."""
    import types
    import antenv
    if "antenv.axon_hooks" in sys.modules:
        return
    mod = types.ModuleType("antenv.axon_hooks")
    mod._hook = None

    def set_axon_ntff_profile_hook(h):
        mod._hook = h

    def get_axon_ntff_profile_hook():
        return mod._hook

    mod.set_axon_ntff_profile_hook = set_axon_ntff_profile_hook
    mod.get_axon_ntff_profile_hook = get_axon_ntff_profile_hook
    sys.modules["antenv.axon_hooks"] = mod
    antenv.axon_hooks = mod
    sys.path.insert(0, "/root/.axon_site")
    from trn_agent_boot.trn_boot import _ntff_profile_via_ctypes
    hook = _ntff_profile_via_ctypes("/opt/axon/libaxon_pjrt.so")
    set_axon_ntff_profile_hook(hook)


def run_traced(inputs, trace_kwargs=None):
    """Traced run for profiling: returns (BassKernelResults, tmpdir)."""
    global _NC_CACHE
    if _NC_CACHE is None:
        _NC_CACHE = _build()
    import tempfile
    _install_ntff_shim()
    in_maps = _make_in_maps(inputs)
    tmpdir = tempfile.mkdtemp(prefix="trace_")
    res = run_bass_kernel_spmd(_NC_CACHE, in_maps, list(range(NCORES)),
                               trace=True, tmpdir=tmpdir,
                               trace_kwargs=trace_kwargs or {})
    return res, tmpdir
